# revision 1
# baseline (speedup 1.0000x reference)
"""Trainium2 Bass kernel for nn_Interpolator (quadratic-form kernel interpolation).

Math (T=8192 targets, C=8192 contexts, D=64, DY=32):
    S = W + W^T
    scores[t,c] = (z_t - z_c)^T W (z_t - z_c)
                = q_tt[t] + q_cc[c] - z_t^T S z_c
    theta = exp(-scores);  out = (theta @ y_context) / theta.sum(-1, keepdim)

q_tt[t] scales whole theta rows and cancels in the normalization -> dropped.
q_cc[c] = 0.5 * z_c^T S z_c is folded into the main matmul contraction:
the stationary operand LC has 128 rows: rows 0..63 = zc^T, rows 64..127 =
0.5*(zc .* (S zc)); the moving operand RT has rows 0..63 = S^T z_t and
rows 64..127 = -1. A single K=128 fp16 matmul then yields cross - q_cc
directly (matmul cost depends only on moving columns, so the fold is free).

Sharding: data-parallel over targets; each of the 8 cores takes T/8 = 1024
targets and the full context set.

Per-core device program — double-buffered score/theta tiles paced by the
ACT engine (the exp over 8.4M scores is the roofline):
  - chunk j: two [128,128]x[128,512] fp16 matmuls fill score tile
    PSC[j%2] (2 PSUM banks each); ONE 1024-wide ACTIVATE produces the
    chunk's bf16 thetas into THS[j%2].
  - mm2 for chunk j is emitted after chunk j+1's score matmuls: o2
    [33,1024] accumulates YA^T @ theta; y_aug col 32 = ones gives the
    denominator row. Dependency tracking is whole-tile for
    write-after-read, so the parity split makes chunk j+2's matmuls wait
    exactly on ACT(j) (the last reader of PSC[j%2]) and ACT(j+2) wait on
    mm2(j) — the intended 2-chunk pipeline, with no false serialization
    against the in-flight chunk.
  - the 16 zs pieces (rows 64..127 of LC) use a separate pool-managed
    PSUM tag: pieces 0..5 in the prelude (overlapping the DMA phase),
    pieces 6..15 every 4th chunk in PE/DVE slack.
  - input DMAs are issued up front across the SP and ACT HWDGE queues
    (z_target + zc on SP, W + y on ACT); zc is split so early chunks can
    start while the rest streams in.
  - a dependency-free matmul burst flips the PE HAM clock-gate to 8/8
    early; a dummy Exp preloads the ACT spline table; the S^T z_t cast
    runs on the otherwise-idle ACT engine, staged in PSC[1] so chunk 0
    never waits on it.
Host: shard/transpose/cast inputs (layout only), concat per-core [33,1024]
outputs, divide numerator rows by the denominator row.
"""

import ml_dtypes
import numpy as np

import concourse.bacc as bacc
import concourse.bass as bass
import concourse.mybir as mybir
import concourse.tile as tile
from concourse.bass_utils import run_bass_kernel_spmd

F32 = mybir.dt.float32
F16 = mybir.dt.float16
BF16 = mybir.dt.bfloat16

T, C, D, DY = 8192, 8192, 64, 32
NCORES = 8
TL = T // NCORES          # 1024 targets per core
NCHUNK = C // 128         # 64 context chunks of 128
NPIECE = C // 512         # 16 zs pieces of 512 contexts
NWARM = 4


def _build_kernel_body(tc: tile.TileContext):
    nc = tc.nc
    Exp = mybir.ActivationFunctionType.Exp

    wwt_d = nc.dram_tensor("wwt", [D, 2 * D], F32, kind="ExternalInput")
    zt_d = nc.dram_tensor("ztt", [D, TL], F16, kind="ExternalInput")
    zcb_d = [
        nc.dram_tensor(f"zcb{b}", [D, 1024], F16, kind="ExternalInput")
        for b in range(8)
    ]
    y_d = nc.dram_tensor("yck", [128, NCHUNK * DY], BF16, kind="ExternalInput")
    out_d = nc.dram_tensor("out", [DY + 1, TL], F32, kind="ExternalOutput")

    with (
        tc.tile_pool(name="sb", bufs=1) as sb,
        tc.tile_pool(name="pp", bufs=1, space="PSUM") as pp,
    ):
        # ---- resident SBUF slabs ----
        LC = sb.tile([128, C], F16, name="lc")
        RT = sb.tile([128, TL], F16, name="rt")
        ZT = sb.tile([D, TL], F16, name="zt")
        YT = sb.tile([128, NCHUNK * DY], BF16, name="yt")
        YA = sb.tile([128, NCHUNK, DY + 1], BF16, name="ya")
        WW = sb.tile([D, 2 * D], F32, name="ww")
        SS = sb.tile([D, D], F16, name="ss")
        SSH = sb.tile([D, D], F16, name="ssh")
        TH0 = sb.tile([128, TL], BF16, name="th0")
        TH1 = sb.tile([128, TL], BF16, name="th1")
        TH2 = sb.tile([128, TL], BF16, name="th2")
        OSB = sb.tile([DY + 1, TL], F32, name="osb")
        WRM = sb.tile([128, 512], BF16, name="wrm")
        EXD = sb.tile([D, 1], F32, name="exd")
        LCS0 = sb.tile([D, 512], F16, name="lcs0")
        LCS1 = sb.tile([D, 512], F16, name="lcs1")
        LCS = [LCS0, LCS1]
        THS = [TH0, TH1, TH2]

        # ---- PSUM: 3 rotating score tiles (6 banks) + 2-bank output
        # accumulator. Everything else (warm-up fills, RT staging, zs
        # pieces) borrows rotation slots; whole-tile WAR tracking makes
        # each borrow wait exactly on the previous tenant's readers. ----
        PS0 = pp.tile([128, TL], F32, tag="ring0", name="ps0")
        PS1 = pp.tile([128, TL], F32, tag="ring1", name="ps1")
        PS2 = pp.tile([128, TL], F32, tag="ring2", name="ps2")
        # output accumulator as two 1-bank tiles so the epilogue can
        # evacuate each target half as soon as its last mm2 stops.
        o2a = pp.tile([DY + 1, 512], F32, tag="o2a", name="o2a")
        o2b = pp.tile([DY + 1, 512], F32, tag="o2b", name="o2b")
        O2 = [o2a, o2b]
        PSC = [PS0, PS1, PS2]

        # ---- input DMAs, split across both HWDGE queues; zc split into
        # 1024-col blocks so early chunks aren't gated on bulk-transfer
        # completion sems even when the pod's DMA bandwidth runs slow.
        # The tiny W rides the ACT queue so it lands first. ----
        nc.sync.dma_start(out=ZT, in_=zt_d.ap())
        for b in range(8):
            nc.sync.dma_start(
                out=LC[:D, b * 1024 : (b + 1) * 1024], in_=zcb_d[b].ap()
            )

        nc.vector.memset(EXD, 0.0)
        nc.scalar.activation(EXD, EXD, Exp)   # exp-table preload
        nc.scalar.dma_start(out=WW, in_=wwt_d.ap())
        half_y = 512
        nc.scalar.dma_start(out=YT[:, :half_y], in_=y_d.ap()[:, :half_y])

        # ---- PE warm-up burst: flips HAM to 8/8 before the loop ----
        nc.vector.memset(WRM, 0.5)
        for i in range(NWARM):
            nc.tensor.matmul(
                PSC[i % 3][:, 0:512], WRM[:, 0:128], WRM, start=True, stop=True
            )

        # ---- DVE prelude chain ----
        nc.vector.memset(RT[D:128, :], -1.0)
        nc.vector.tensor_add(SS, WW[:, 0:D], WW[:, D : 2 * D])   # fp16 S
        nc.vector.tensor_scalar_mul(SSH, SS, 0.5)                # fp16 S/2

        # ---- RT rows 0..63 = S^T zt, staged in PS2 (chunk 2's tile) so
        # chunks 0/1 never wait; cast h0 on the idle ACT engine and h1 on
        # the DVE, in parallel ----
        for h in range(2):
            sl = slice(h * 512, (h + 1) * 512)
            nc.tensor.matmul(PS2[:D, sl], SS, ZT[:, sl], start=True, stop=True)
        nc.scalar.copy(RT[:D, 0:512], PS2[:D, 0:512])
        nc.vector.tensor_copy(RT[:D, 512:1024], PS2[:D, 512:1024])
        nc.scalar.dma_start(out=YT[:, half_y:], in_=y_d.ap()[:, half_y:])

        # zs piece: LC rows 64..127 = 0.5*(zc .* S zc) over a column
        # slice, staged in region [512:1024] of a borrowed rotation tile.
        def zs_mm(sl, tl):
            w = sl.stop - sl.start
            nc.tensor.matmul(PSC[tl][:D, 512 : 512 + w], SSH, LC[:D, sl],
                             start=True, stop=True)

        # In-loop pieces write the product into an SBUF staging tile and
        # an SBUF->SBUF DMA lands it in LC: the loop's matmuls then never
        # inherit a wait on the (late) DVE mul, only on the DMA, which
        # completes chunks before its columns are read.
        def zs_mul_dma(sl, tl, st):
            w = sl.stop - sl.start
            nc.vector.tensor_mul(LCS[st][:, 0:w], PSC[tl][:D, 512 : 512 + w],
                                 LC[:D, sl])
            nc.sync.dma_start(out=LC[D:128, sl], in_=LCS[st][:, 0:w])

        def zs_piece(sl, tl):
            w = sl.stop - sl.start
            zs_mm(sl, tl)
            nc.vector.tensor_mul(LC[D:128, sl], PSC[tl][:D, 512 : 512 + w],
                                 LC[:D, sl])

        # y_aug piece q (16 chunks): [128, 16, 33]; col 32 = 1.0
        nc.vector.memset(YA[:, :, DY : DY + 1], 1.0)
        qy = NCHUNK // 4 * DY

        def ya_piece(q):
            nc.vector.tensor_copy(
                YA[:, q * 16 : (q + 1) * 16, 0:DY],
                YT[:, q * qy : (q + 1) * qy].rearrange("p (j d) -> p j d", d=DY),
            )

        # prelude zs: a 128-col mini-piece unblocks chunk 0 the moment
        # zc block 0 lands; two more cover chunks 1..7.
        zs_piece(slice(0, 128), 0)
        zs_piece(slice(128, 512), 1)
        zs_piece(slice(512, 1024), 2)
        ya_piece(0)

        # ---- main loop over 64 context chunks ----
        # zs piece k (k>=2) lands at chunk 3(k-2)+1, needed by chunk 4k;
        # ya pieces 1..3 ride along at chunks 3, 7, 11.
        ready = []     # chunks whose thetas are exp'd, mm2 pending

        def emit_mm2s():
            for j in ready:
                for h in range(2):
                    nc.tensor.matmul(
                        O2[h][:, :],
                        YA[:, j, :],
                        THS[j % 3][:, h * 512 : (h + 1) * 512],
                        start=(j == 0),
                        stop=(j == NCHUNK - 1),
                    )
            ready.clear()

        for j in range(NCHUNK):
            P = PSC[j % 3]
            lhsT = LC[:, j * 128 : (j + 1) * 128]
            for h in range(2):
                nc.tensor.matmul(
                    P[:, h * 512 : (h + 1) * 512],
                    lhsT,
                    RT[:, h * 512 : (h + 1) * 512],
                    start=True,
                    stop=True,
                )
            # zs piece here, borrowing tile (j+2)%3 = chunk (j-1)'s: its
            # staging matmul is co-gated with mm2(j-1) on ACT(j-1), and
            # the tile's next writer (chunk j+2's matmuls) starts two ACT
            # windows after the DVE mul fires -> no pipeline stall. The
            # mul lands in SBUF staging; an SBUF->SBUF DMA (tracked
            # subtile-precisely) carries it into LC rows 64..127.
            if j % 3 == 1 and 2 + (j - 1) // 3 < NPIECE:
                k = 2 + (j - 1) // 3
                ksl = slice(512 * k, 512 * (k + 1))
                zs_mm(ksl, (j + 2) % 3)
                zs_mul_dma(ksl, (j + 2) % 3, k % 2)
            if j == NCHUNK - 1:
                # split the last exp so mm2(63,h0) and the h0 evacuation
                # start half a window earlier
                for h in range(2):
                    sl = slice(h * 512, (h + 1) * 512)
                    nc.scalar.activation(THS[j % 3][:, sl], P[:, sl], Exp)
            else:
                nc.scalar.activation(THS[j % 3], P, Exp)
            emit_mm2s()
            ready.append(j)
            if j in (3, 7, 11):
                ya_piece((j + 1) // 4)
        emit_mm2s()

        # ---- epilogue: evacuate each half as soon as its mm2 stops
        # (ACT + DVE in parallel), then one out-DMA ----
        nc.scalar.copy(OSB[:, 0:512], o2a)
        nc.vector.tensor_copy(OSB[:, 512:1024], o2b)
        nc.sync.dma_start(out=out_d.ap(), in_=OSB)


_CACHED = None


def _get_nc():
    global _CACHED
    if _CACHED is None:
        nc = bacc.Bacc(
            "TRN2",
            target_bir_lowering=False,
            debug=False,
            enable_asserts=False,
        )
        with tile.TileContext(nc) as tc:
            _build_kernel_body(tc)
        nc.compile()
        _CACHED = nc
    return _CACHED


def make_in_maps(z_context, y_context, z_target, W):
    """Host-side layout prep (transpose/reshape/cast only) + sharding."""
    z_context = np.asarray(z_context, dtype=np.float32)
    y_context = np.asarray(y_context, dtype=np.float32)
    z_target = np.asarray(z_target, dtype=np.float32)
    W = np.asarray(W, dtype=np.float32)

    zcT = np.ascontiguousarray(z_context.T.astype(np.float16))  # [64, 8192]
    zcb = [
        np.ascontiguousarray(zcT[:, b * 1024 : (b + 1) * 1024])
        for b in range(8)
    ]
    # chunk j partition p holds context j*128+p:
    # yck[p, j*DY+d] = y_context[j*128+p, d]
    yck = np.ascontiguousarray(
        y_context.reshape(NCHUNK, 128, DY).transpose(1, 0, 2).reshape(
            128, NCHUNK * DY
        )
    ).astype(ml_dtypes.bfloat16)
    wwt = np.ascontiguousarray(np.concatenate([W, W.T], axis=1))  # [64, 128]

    in_maps = []
    for i in range(NCORES):
        ztT = np.ascontiguousarray(
            z_target[i * TL : (i + 1) * TL].T.astype(np.float16)
        )
        m = {"wwt": wwt, "ztt": ztT, "yck": yck}
        for b in range(8):
            m[f"zcb{b}"] = zcb[b]
        in_maps.append(m)
    return in_maps


def postprocess(results):
    """Gather per-core [33, TL] outputs -> full (T, DY) normalized output."""
    allT = np.concatenate([r["out"].T for r in results], axis=0)  # [T, 33]
    return (allT[:, :DY] / allT[:, DY : DY + 1]).astype(np.float32)


def run(in_maps, **kwargs):
    nc = _get_nc()
    return run_bass_kernel_spmd(nc, in_maps, core_ids=list(range(NCORES)), **kwargs)


def kernel(z_context, y_context, z_target, W):
    in_maps = make_in_maps(z_context, y_context, z_target, W)
    res = run(in_maps)
    return postprocess(res.results)

